# revision 2
# baseline (speedup 1.0000x reference)
"""Attention via the x(WqWk^T)x^T factorization + pair-wise V AllGather.

Key idea vs the v3 baseline: scores = Q K^T = x (Wq Wk^T) x^T, so with
M = Wq Wk^T precomputed on host (768x768 weight folding), the kernel
never forms Q or K at all.  Per core it computes t = x_half M (same cost
as the old Q-half projection) and contracts scoresT directly against the
x^T slabs (xh) that are already local -- eliminating the old A3 phase
that redundantly computed the full-sequence K^T on every core (~31us of
PE time, 144 matmuls).

V is unchanged: each core computes V' for its own half from xq and the
pair exchanges halves with one 2-rank AllGather (trigger ~21us, landing
~78us -- measured).  Phase B emits [all scoresT+exp] then [all out
runs]; the first vp touch is at out-phase start (~79us), just after the
gather readback lands, so the collective stays off the critical path.

Output is stored fp16 (host upcasts); halves the output DMA tail.
Numerics (numpy sim of this exact quantization chain): 1.19e-3 max rel.
"""

import numpy as np

import concourse.bass as bass
import concourse.mybir as mybir
import concourse.tile as tile
from concourse import bacc
from concourse.bass_utils import run_bass_kernel_spmd

N_CORES = 8
B, N, D, OUT = 4, 2048, 768, 768
NQ = N // 2
P = 128
DC = D // P
KC = N // P
HKC = KC // 2  # k-chunks per half
F32 = mybir.dt.float32
FP16 = mybir.dt.float16
PAIRS = [[0, 1], [2, 3], [4, 5], [6, 7]]

Q_BLOCKS = [(0, 512), (512, 512)]


def build_attention_nc():
    nc = bacc.Bacc("TRN2", target_bir_lowering=False, debug=False)
    xh = nc.dram_tensor("xh", [D, N], FP16, kind="ExternalInput")
    xq = nc.dram_tensor("xq", [D, NQ], FP16, kind="ExternalInput")
    wv = nc.dram_tensor("wv", [D, OUT], FP16, kind="ExternalInput")
    m = nc.dram_tensor("m", [D, D], FP16, kind="ExternalInput")
    out = nc.dram_tensor("out", [NQ, OUT], FP16, kind="ExternalOutput")

    with tile.TileContext(nc) as tc:
        with (
            tc.tile_pool(name="persist", bufs=1) as persist,
            tc.tile_pool(name="dpool", bufs=1, space="DRAM") as dpool,
        ):
            tt = persist.tile([P, DC, NQ], FP16)  # tT[j,q] = (x_half M)^T
            vp = persist.tile([P, KC, OUT + 2], FP16)  # V' physical order
            kslab_tiles = [
                persist.tile([P, DC, 512], FP16, name=f"kslab{s}")
                for s in range(4)
            ]

            vpb_in = dpool.tile([P, HKC, OUT + 2], FP16)
            vpb_out = dpool.tile([2, P, HKC, OUT + 2], FP16)

            ones_sc = persist.tile([P, 1], F32, name="ones_sc")
            nc.vector.memset(ones_sc, 1.0)
            zero_sc = persist.tile([P, 1], F32, name="zero_sc")
            nc.vector.memset(zero_sc, 0.0)

            with (
                tc.tile_pool(name="slabs", bufs=2) as slabs,
                tc.tile_pool(name="psa", bufs=7, space="PSUM") as psa,
                tc.tile_pool(name="wpool", bufs=1) as wpool,
                tc.tile_pool(name="stage", bufs=4) as stage,
            ):
                wv_sb = wpool.tile([P, DC, OUT], FP16)
                m_sb = wpool.tile([P, DC, D], FP16)

                # HAM warmup while the first DMAs fly
                warm = wpool.tile([P, 512], FP16, name="warm")
                nc.vector.memset(warm, 1.0)
                wps = psa.tile([P, 512], F32, name="wps", bufs=1)
                for i in range(11):
                    nc.tensor.matmul(
                        wps, warm[:, 0:P], warm, start=(i == 0), stop=(i == 10)
                    )

                # DMAs: wv/xq-slab0 first (V feeds the gather, so it runs
                # earliest), then xq-slab1, M (tT input), and the 4 xh
                # slabs (only needed by scoresT at ~37us)
                qslab_tiles = []
                for s in range(2):
                    qslab = slabs.tile(
                        [P, DC, 512], FP16, tag="slab", name=f"qslab{s}"
                    )
                    src = xq[:, s * 512 : (s + 1) * 512]
                    if s == 0:
                        for dc in range(DC):
                            nc.gpsimd.dma_start(
                                out=wv_sb[:, dc, :],
                                in_=wv[dc * P : (dc + 1) * P, :],
                            )
                            nc.sync.dma_start(
                                out=qslab[:, dc, :],
                                in_=src[dc * P : (dc + 1) * P, :],
                            )
                    else:
                        nc.sync.dma_start(
                            out=qslab,
                            in_=src.rearrange("(dc p) n -> p dc n", p=P),
                        )
                    qslab_tiles.append(qslab)
                for dc in range(DC):
                    nc.sync.dma_start(
                        out=m_sb[:, dc, :], in_=m[dc * P : (dc + 1) * P, :]
                    )
                for s in range(4):
                    nc.sync.dma_start(
                        out=kslab_tiles[s],
                        in_=xh[:, s * 512 : (s + 1) * 512].rearrange(
                            "(dc p) n -> p dc n", p=P
                        ),
                    )

                # ---- A1: V' half (earliest -> feeds the gather) ----
                for s in range(2):
                    slab = qslab_tiles[s]
                    for j in range(4):
                        kc = s * 4 + j
                        ps1 = psa.tile([P, 512], F32, tag="psa")
                        ps2 = psa.tile([P, 512], F32, tag="psa")
                        # interleaved: consecutive matmuls share the same
                        # stationary slab slice, so the second weight load
                        # overlaps/elides (measured 187 vs 214 ns/matmul)
                        for dc in range(DC):
                            nc.tensor.matmul(
                                ps1[:, 0:384],
                                slab[:, dc, j * P : (j + 1) * P],
                                wv_sb[:, dc, 0:384],
                                start=(dc == 0),
                                stop=(dc == DC - 1),
                            )
                            nc.tensor.matmul(
                                ps2[:, 0:384],
                                slab[:, dc, j * P : (j + 1) * P],
                                wv_sb[:, dc, 384:OUT],
                                start=(dc == 0),
                                stop=(dc == DC - 1),
                            )
                        vst = stage.tile([P, OUT + 2], FP16, tag="vst", bufs=9)
                        nc.vector.tensor_copy(vst[:, 0:384], ps1[:, 0:384])
                        nc.vector.tensor_copy(vst[:, 384:OUT], ps2[:, 0:384])
                        nc.vector.tensor_copy(vst[:, OUT : OUT + 1], ones_sc)
                        nc.vector.tensor_copy(
                            vst[:, OUT + 1 : OUT + 2], zero_sc
                        )
                        nc.gpsimd.dma_start(out=vpb_in[:, kc, :], in_=vst)
                nc.gpsimd.collective_compute(
                    "AllGather",
                    mybir.AluOpType.bypass,
                    replica_groups=PAIRS,
                    ins=[vpb_in.opt()],
                    outs=[vpb_out.opt()],
                )
                # NOT on the scalar ring: the ACT sequencer is busy with
                # the exp activations when the gather lands (measured
                # 4.4us stall).  4 quarter-DMAs so the out runs can start
                # on the first quarter while the rest stream in.
                for h in range(2):
                    for q4 in range(0, HKC, 4):
                        nc.sync.dma_start(
                            out=vp[:, h * HKC + q4 : h * HKC + q4 + 4, :],
                            in_=vpb_out[h][:, q4 : q4 + 4, :],
                        )

                # ---- A2: tT half (local): t = x_half M ----
                for s in range(2):
                    slab = qslab_tiles[s]
                    for jc in range(DC):
                        ps = psa.tile([P, 512], F32, tag="psa")
                        for dc in range(DC):
                            nc.tensor.matmul(
                                ps,
                                m_sb[:, dc, jc * P : (jc + 1) * P],
                                slab[:, dc, :],
                                start=(dc == 0),
                                stop=(dc == DC - 1),
                            )
                        nc.vector.tensor_copy(
                            tt[:, jc, s * 512 : (s + 1) * 512], ps
                        )

            # ---- phase B: all scoresT runs, then all out runs ----
            with (
                tc.tile_pool(name="expp", bufs=33) as expp,
                tc.tile_pool(name="obp", bufs=3) as obp,
                tc.tile_pool(name="smallp", bufs=4) as smallp,
                tc.tile_pool(name="ps_sc", bufs=2, space="PSUM") as ps_sc,
                tc.tile_pool(name="ps_out", bufs=3, space="PSUM") as ps_out,
            ):
                ets = {}
                for bi, (q0, qb) in enumerate(Q_BLOCKS):
                    for kc in range(KC):
                        s, j = kc // 4, kc % 4
                        kslab = kslab_tiles[s]
                        st = ps_sc.tile([P, 512], F32, tag="sc")
                        for dc in range(DC):
                            nc.tensor.matmul(
                                st[:, 0:qb],
                                kslab[:, dc, j * P : (j + 1) * P],
                                tt[:, dc, q0 : q0 + qb],
                                start=(dc == 0),
                                stop=(dc == DC - 1),
                            )
                        et = expp.tile(
                            [P, 512], FP16, tag="exp", name=f"et{bi}_{kc}"
                        )
                        nc.scalar.activation(
                            et[:, 0:qb],
                            st[:, 0:qb],
                            mybir.ActivationFunctionType.Exp,
                            scale=0.125,
                        )
                        ets[(bi, kc)] = et
                for bi, (q0, qb) in enumerate(Q_BLOCKS):
                    nqc = qb // P
                    # per q-chunk: one 16-matmul run into each PSUM bank
                    # (alternating the two banks of a tile every matmul
                    # makes the PE micro-idle); normalize right after each
                    # chunk so bufs=3 never stalls the 4-chunk blocks
                    for j in range(nqc):
                        ops = ps_out.tile(
                            [P, OUT + 2], F32, tag="out", name=f"outps{bi}_{j}"
                        )
                        for kc in range(KC):
                            nc.tensor.matmul(
                                ops[:, 0:512],
                                ets[(bi, kc)][:, j * P : (j + 1) * P],
                                vp[:, kc, 0:512],
                                start=(kc == 0),
                                stop=(kc == KC - 1),
                            )
                        for kc in range(KC):
                            nc.tensor.matmul(
                                ops[:, 512 : OUT + 2],
                                ets[(bi, kc)][:, j * P : (j + 1) * P],
                                vp[:, kc, 512 : OUT + 2],
                                start=(kc == 0),
                                stop=(kc == KC - 1),
                            )
                        recip = smallp.tile([P, 1], F32, tag="recip")
                        nc.vector.reciprocal(recip, ops[:, OUT : OUT + 1])
                        ob = obp.tile([P, OUT], FP16, tag="ob")
                        nc.vector.tensor_scalar_mul(ob, ops[:, 0:OUT], recip)
                        nc.sync.dma_start(
                            out=out[q0 + j * P : q0 + (j + 1) * P, :], in_=ob
                        )
    nc.finalize()
    return nc


_NC_CACHE = None


def _get_nc():
    global _NC_CACHE
    if _NC_CACHE is None:
        _NC_CACHE = build_attention_nc()
    return _NC_CACHE


def make_in_maps(x, kernel):
    x = np.asarray(x, dtype=np.float32)
    w16 = np.asarray(kernel, dtype=np.float32).astype(np.float16)
    m = (w16[0].astype(np.float32) @ w16[1].astype(np.float32).T).astype(
        np.float16
    )
    wv = np.ascontiguousarray(w16[2])
    in_maps = []
    for core in range(N_CORES):
        b, half = core // 2, core % 2
        xt16 = x[b].T.astype(np.float16)
        xh = np.ascontiguousarray(xt16)
        xq = np.ascontiguousarray(xt16[:, half * NQ : (half + 1) * NQ])
        in_maps.append({"xh": xh, "xq": xq, "wv": wv, "m": m})
    return in_maps


def assemble_output(results):
    out = np.empty((B, N, OUT), dtype=np.float32)
    for core in range(N_CORES):
        b, half = core // 2, core % 2
        out[b, half * NQ : (half + 1) * NQ, :] = results[core]["out"].astype(
            np.float32
        )
    return out


def run_on_hw(x, kernel, trace=False):
    nc = _get_nc()
    res = run_bass_kernel_spmd(
        nc, make_in_maps(x, kernel), list(range(N_CORES)), trace=trace
    )
    return assemble_output(res.results), res


def kernel(x, kernel):
    out, _ = run_on_hw(x, kernel, trace=False)
    return out
